# revision 6
# baseline (speedup 1.0000x reference)
"""Trainium2 Bass kernel for nn_DL_R_sum_MRC (MIMO MRC rate-sum loss).

Math (per batch b, RB i, subcarrier j, user k), derived from reference:
  V[c,t]   : unnormalized complex precoder (from y_pred), per (b, i)
  N2[c]    = sum_t |V[c,t]|^2           (normalization folded into the logs)
  hv[r,c]  = sum_t H_k[t,r] * V[c,t]    (complex, unnormalized)
  HF = hv[:,k], G = hv[:,1-k]
  q_u  = sum_r |HF_r|^2
  u_u  = sum_r conj(HF_r) * G_r
  DEN  = N2_k * (sigma * q_u * N2_kb + |u_u|^2 * P_kb)
  NUM  = DEN + q_u^2 * P_k * N2_kb
  rate = (ln NUM - ln DEN) / ln 2
  loss = -sum rate / (B * 52)

Sharding: pure data-parallel over batch, 8 NeuronCores x 512 batch.
Each core reduces its rates to a [128, NCHUNK] partial-sum tile; host sums.

Complex dot products via 3-mul Karatsuba (per H-element h=a+ib, V-elem
v=c+id): S1 = sum vre*(hre+him), S2 = sum hre*(vim-vre),
S3 = sum him*(vre+vim); hv_re = S1-S3, hv_im = S1+S2.

On-chip layouts (batch in partitions, 128 per chunk; offsets in elements):
  hraw (DMA):  (sc26, t32, r2, e2)   sc*128 + t*4 + r*2 + e      f32 half
  hk[k]:       (sc52, r2, m3, t32)   sc*192 + r*96 + m*32 + t    bf16
               m in {0: re, 1: im, 2: re+im}
  vk:          (s3, c2, i13, t32)    s*832 + c*416 + i*32 + t    bf16
               s in {0: vre, 1: vim-vre, 2: vre+vim}
  vsq:         (s'2, c, i, t)        mirror of vk s1..s2          f32 (x0.5)
  pr:          (q=(i,j,r,c), t32)    q*32 + t                     bf16
  pl1..pl4:    (q, t16/8/4/2)
  sk[k]:       (m3, q208)            m*208 + q                    bf16
  hvall:       (part2, k2, i, jr, c) part*416 + k*208 + i*16+j*4.. wait
               q = i*16 + j*4 + r*2 + c ; hv uses (i, jr=(j,r), c) = q   f32
"""

import math
import sys

import numpy as np

sys.path.insert(0, "/opt/trn_rl_repo")

B_FULL = 4096
N_CORES = 8
NB = B_FULL // N_CORES  # 512 batch per core
P = 128                 # partitions per chunk
NCHUNK = NB // P        # 4 chunks
SIGMA = 0.1
NRB = 13
NSC = 52

H_FREE = NSC * 32 * 2 * 2   # 6656
Y_FREE = 64 * NRB * 2       # 1664
P_FREE = NRB * 2            # 26: (i, c)

_TRACE = {"on": False, "result": None}

# engine assignment: "v" = DVE (vector), "g" = GpSimd (Pool)
MUL_ENG = {(0, 0): "v", (0, 1): "v", (1, 0): "v", (1, 1): "v",
           (2, 0): "g", (2, 1): "g"}
TREE_ENG = {(0, 0): "v", (0, 1): "v", (1, 0): "v", (1, 1): "g",
            (2, 0): "g", (2, 1): "v"}
HSUM_ENG = "v"
VPREP_ENG = "v"
UMUL_ENG = "g"


def _ap(x, off, dims):
    """View of tile/dram AP `x` at element offset `off` with free dims [[step, count], ...]."""
    import concourse.bass as bass

    return bass.AP(tensor=x.tensor, offset=x.offset + off, ap=[list(x.ap[0])] + dims)


def _build(nc, repeat=1, parts="all"):
    from contextlib import ExitStack

    import concourse.tile as tile
    from concourse import mybir

    f32 = mybir.dt.float32
    bf16 = mybir.dt.bfloat16
    Alu = mybir.AluOpType
    Act = mybir.ActivationFunctionType
    Ax = mybir.AxisListType

    h1d = nc.dram_tensor("h1", [NB, H_FREE], f32, kind="ExternalInput").ap()
    h2d = nc.dram_tensor("h2", [NB, H_FREE], f32, kind="ExternalInput").ap()
    yd = nc.dram_tensor("yp", [NB, Y_FREE], f32, kind="ExternalInput").ap()
    pd = nc.dram_tensor("pm", [NB, P_FREE], f32, kind="ExternalInput").ap()
    outd = nc.dram_tensor("partial", [P, NCHUNK], f32, kind="ExternalOutput").ap()

    def eng(which):
        return nc.vector if which == "v" else nc.gpsimd

    with tile.TileContext(nc) as tc, ExitStack() as ctx:
        hpool = ctx.enter_context(tc.tile_pool(name="hpool", bufs=2))
        hkpool = ctx.enter_context(tc.tile_pool(name="hkpool", bufs=2))
        vpool = ctx.enter_context(tc.tile_pool(name="vpool", bufs=2))
        prpool = ctx.enter_context(tc.tile_pool(name="prpool", bufs=2))
        plpool = ctx.enter_context(tc.tile_pool(name="plpool", bufs=2))
        skpool = ctx.enter_context(tc.tile_pool(name="skpool", bufs=2))
        epool = ctx.enter_context(tc.tile_pool(name="epool", bufs=2))
        persist = ctx.enter_context(tc.tile_pool(name="persist", bufs=1))

        racc = persist.tile([P, NCHUNK], f32)

        for ch in [c for _ in range(repeat) for c in range(NCHUNK)]:
            b0 = ch * P
            bsl = slice(b0, b0 + P)

            yt = vpool.tile([P, Y_FREE], f32, tag="yt")
            nc.sync.dma_start(out=yt, in_=yd[bsl, :])
            pt = vpool.tile([P, P_FREE], f32, tag="pt")
            nc.sync.dma_start(out=pt, in_=pd[bsl, :])

            # ---- H relayout + cast: hraw (sc,t,r,e) -> hk (sc, r, m, t) ----
            hks = []
            for k, hd in ((0, h1d), (1, h2d)):
                hk = hkpool.tile([P, NSC * 192], bf16, tag=f"hk{k}")
                hks.append(hk)
                for half in range(2):
                    off = half * 26 * 128
                    hraw = hpool.tile([P, H_FREE // 2], f32, tag="hraw",
                                      name=f"hraw{k}{half}")
                    nc.sync.dma_start(out=hraw, in_=hd[bsl, off:off + 3328])
                    for e in range(2):
                        nc.scalar.copy(
                            _ap(hk, half * 26 * 192 + e * 32,
                                [[192, 26], [96, 2], [1, 32]]),
                            _ap(hraw, e, [[128, 26], [2, 2], [4, 32]]),
                        )
                # hsum slot m=2: re + im
                eng(HSUM_ENG).tensor_add(
                    _ap(hk, 64, [[192, NSC], [96, 2], [1, 32]]),
                    _ap(hk, 0, [[192, NSC], [96, 2], [1, 32]]),
                    _ap(hk, 32, [[192, NSC], [96, 2], [1, 32]]),
                )

            # ---- V prep: vk (s, c, i, t) ----
            vk = vpool.tile([P, 3 * 832], bf16, tag="vk")
            vdim = [[416, 2], [32, NRB], [1, 32]]
            # s0: vre
            nc.scalar.copy(_ap(vk, 0, vdim),
                           _ap(yt, 0, [[832, 2], [2, NRB], [26, 32]]))
            yre = _ap(yt, 0, [[832, 2], [2, NRB], [26, 32]])
            yim = _ap(yt, 1, [[832, 2], [2, NRB], [26, 32]])
            # s1: vim - vre ; s2: vre + vim
            eng(VPREP_ENG).tensor_sub(_ap(vk, 832, vdim), yim, yre)
            eng(VPREP_ENG).tensor_add(_ap(vk, 1664, vdim), yre, yim)

            # ---- N2[c,i] = sum_t vre^2+vim^2 = sum_{s',t} vk[s1..s2]^2 / 2
            vsq = persist.tile([P, 1664], f32, tag="vsq")
            nc.scalar.activation(vsq, _ap(vk, 832, [[1, 1664]]), Act.Square,
                                 scale=float(1.0 / math.sqrt(2.0)))
            n2s = epool.tile([P, 52], f32, tag="n2s")  # (s', ci)
            nc.vector.tensor_reduce(
                out=_ap(n2s, 0, [[1, 52]]),
                in_=_ap(vsq, 0, [[832, 2], [32, 26], [1, 32]]),
                axis=Ax.X, op=Alu.add)
            n2 = epool.tile([P, 26], f32, tag="n2")  # (c, i): c*13+i
            nc.vector.tensor_add(n2, _ap(n2s, 0, [[1, 26]]),
                                 _ap(n2s, 26, [[1, 26]]))

            if parts == "dmaonly":
                nc.vector.tensor_copy(_ap(racc, ch, [[1, 1]]),
                                      _ap(hks[0], 0, [[1, 1]]))
                continue

            # ---- products + tree per (k, m-stream) ----
            # stream m: (h-slot, v-slot): m0: (hsum=2, vre=0) -> S1
            #           m1: (hre=0, vd1=1) -> S2 ; m2: (him=1, vd2=2) -> S3
            hvall = epool.tile([P, 832], f32, tag="hvall")
            sks = []
            for k in range(2):
                hk = hks[k]
                sk = skpool.tile([P, 3 * 208], bf16, tag=f"sk{k}")
                sks.append(sk)
                for m, (hm, vs) in enumerate(((2, 0), (0, 1), (1, 2))):
                    me = eng(MUL_ENG[(m, k)])
                    te = eng(TREE_ENG[(m, k)])
                    pr = prpool.tile([P, 6656], bf16, tag="pr")
                    for r in range(2):
                        for c in range(2):
                            me.tensor_mul(
                                _ap(pr, r * 64 + c * 32,
                                    [[512, NRB], [128, 4], [1, 32]]),
                                _ap(hk, r * 96 + hm * 32,
                                    [[768, NRB], [192, 4], [1, 32]]),
                                _ap(vk, vs * 832 + c * 416,
                                    [[32, NRB], [0, 4], [1, 32]]),
                            )
                    pl1 = plpool.tile([P, 3328], bf16, tag="pl1")
                    te.tensor_add(_ap(pl1, 0, [[16, 208], [1, 16]]),
                                  _ap(pr, 0, [[32, 208], [1, 16]]),
                                  _ap(pr, 16, [[32, 208], [1, 16]]))
                    pl2 = plpool.tile([P, 1664], bf16, tag="pl2")
                    te.tensor_add(_ap(pl2, 0, [[8, 208], [1, 8]]),
                                  _ap(pl1, 0, [[16, 208], [1, 8]]),
                                  _ap(pl1, 8, [[16, 208], [1, 8]]))
                    pl3 = plpool.tile([P, 832], bf16, tag="pl3")
                    te.tensor_add(_ap(pl3, 0, [[4, 208], [1, 4]]),
                                  _ap(pl2, 0, [[8, 208], [1, 4]]),
                                  _ap(pl2, 4, [[8, 208], [1, 4]]))
                    pl4 = plpool.tile([P, 416], bf16, tag="pl4")
                    te.tensor_add(_ap(pl4, 0, [[2, 208], [1, 2]]),
                                  _ap(pl3, 0, [[4, 208], [1, 2]]),
                                  _ap(pl3, 2, [[4, 208], [1, 2]]))
                    te.tensor_add(_ap(sk, m * 208, [[1, 208]]),
                                  _ap(pl4, 0, [[2, 208]]),
                                  _ap(pl4, 1, [[2, 208]]))
                # combine: hv_re = S1 - S3 -> part 0; hv_im = S1 + S2 -> part 1
                nc.vector.tensor_sub(_ap(hvall, k * 208, [[1, 208]]),
                                     _ap(sk, 0, [[1, 208]]),
                                     _ap(sk, 416, [[1, 208]]))
                nc.vector.tensor_add(_ap(hvall, 416 + k * 208, [[1, 208]]),
                                     _ap(sk, 0, [[1, 208]]),
                                     _ap(sk, 208, [[1, 208]]))

            if parts == "prodonly":
                nc.vector.tensor_copy(_ap(racc, ch, [[1, 1]]),
                                      _ap(hvall, 0, [[1, 1]]))
                continue

            # ---- epilogue per user k ----
            rsum = epool.tile([P, 52], f32, tag="rsum")
            for k in range(2):
                kb = 1 - k
                ijr = [[16, 13], [2, 8]]  # (i, jr) views into hvall
                hfre = _ap(hvall, k * 208 + k, ijr)
                hfim = _ap(hvall, 416 + k * 208 + k, ijr)
                gre = _ap(hvall, k * 208 + kb, ijr)
                gim = _ap(hvall, 416 + k * 208 + kb, ijr)

                t1 = epool.tile([P, 104], f32, tag="t1")
                t2 = epool.tile([P, 104], f32, tag="t2")
                rfold_a = [[2, 52]]          # (ij) at r=0, stride 2
                red_out = [[1, 52]]          # (ij)

                # q_u = sum_r hfre^2 + hfim^2
                nc.scalar.activation(t1, hfre, Act.Square)
                nc.scalar.activation(t2, hfim, Act.Square)
                nc.vector.tensor_add(t1, t1, t2)
                qu = epool.tile([P, 52], f32, tag="qu")
                nc.vector.tensor_add(_ap(qu, 0, red_out),
                                     _ap(t1, 0, rfold_a), _ap(t1, 1, rfold_a))
                # u_re = sum_r hfre*gre + hfim*gim
                eng(UMUL_ENG).tensor_mul(t1, hfre, gre)
                eng(UMUL_ENG).tensor_mul(t2, hfim, gim)
                nc.vector.tensor_add(t1, t1, t2)
                ure = epool.tile([P, 52], f32, tag="ure")
                nc.vector.tensor_add(_ap(ure, 0, red_out),
                                     _ap(t1, 0, rfold_a), _ap(t1, 1, rfold_a))
                # u_im = sum_r hfre*gim - hfim*gre
                eng(UMUL_ENG).tensor_mul(t1, hfre, gim)
                eng(UMUL_ENG).tensor_mul(t2, hfim, gre)
                nc.vector.tensor_sub(t1, t1, t2)
                uim = epool.tile([P, 52], f32, tag="uim")
                nc.vector.tensor_add(_ap(uim, 0, red_out),
                                     _ap(t1, 0, rfold_a), _ap(t1, 1, rfold_a))
                # |u|^2
                uu2 = epool.tile([P, 52], f32, tag="uu2")
                nc.scalar.activation(ure, ure, Act.Square)
                nc.scalar.activation(uim, uim, Act.Square)
                nc.vector.tensor_add(uu2, ure, uim)

                bk = _ap(n2, 13 * k, [[1, 13], [0, 4]])
                bkb = _ap(n2, 13 * kb, [[1, 13], [0, 4]])
                pk = _ap(pt, k, [[2, 13], [0, 4]])
                pkb = _ap(pt, kb, [[2, 13], [0, 4]])

                den = epool.tile([P, 52], f32, tag="den")
                num = epool.tile([P, 52], f32, tag="num")
                # den = bk * (sigma*qu*bkb + uu2*pkb)
                nc.vector.scalar_tensor_tensor(
                    out=den, in0=qu, scalar=SIGMA, in1=bkb,
                    op0=Alu.mult, op1=Alu.mult)
                nc.vector.tensor_mul(num, uu2, pkb)  # num as scratch
                nc.vector.tensor_add(den, den, num)
                nc.vector.tensor_mul(den, den, bk)
                # num = den + qu^2 * pk * bkb
                nc.vector.tensor_mul(num, qu, qu)
                nc.vector.tensor_mul(num, num, pk)
                nc.vector.tensor_mul(num, num, bkb)
                nc.vector.tensor_add(num, num, den)

                nc.scalar.activation(den, den, Act.Ln)
                nc.scalar.activation(num, num, Act.Ln)
                if k == 0:
                    nc.vector.tensor_sub(rsum, num, den)
                else:
                    nc.vector.tensor_sub(num, num, den)
                    nc.vector.tensor_add(rsum, rsum, num)

            nc.vector.tensor_reduce(
                out=_ap(racc, ch, [[1, 1]]),
                in_=rsum, axis=Ax.X, op=Alu.add)

        nc.sync.dma_start(out=outd, in_=racc)

    return nc


def _make_program(repeat=1):
    from concourse import bacc

    nc = bacc.Bacc("TRN2", target_bir_lowering=False, debug=False,
                   num_devices=N_CORES)
    _build(nc, repeat=repeat)
    nc.compile()
    return nc


def kernel(H_dl_RB_1, H_dl_RB_2, P_marix, y_pred):
    from concourse.bass_utils import run_bass_kernel_spmd

    h1 = np.ascontiguousarray(np.asarray(H_dl_RB_1, dtype=np.float32)).reshape(B_FULL, H_FREE)
    h2 = np.ascontiguousarray(np.asarray(H_dl_RB_2, dtype=np.float32)).reshape(B_FULL, H_FREE)
    yp = np.ascontiguousarray(np.asarray(y_pred, dtype=np.float32)).reshape(B_FULL, Y_FREE)
    pm = np.ascontiguousarray(np.asarray(P_marix, dtype=np.float32)).reshape(B_FULL, P_FREE)

    nc = _make_program()
    in_maps = []
    for c in range(N_CORES):
        s = slice(c * NB, (c + 1) * NB)
        in_maps.append({"h1": h1[s], "h2": h2[s], "yp": yp[s], "pm": pm[s]})

    res = run_bass_kernel_spmd(nc, in_maps, list(range(N_CORES)),
                               trace=_TRACE["on"])
    _TRACE["result"] = res
    total = np.float64(0.0)
    for r in res.results:
        total += np.float64(r["partial"].astype(np.float64).sum())
    loss = -total / (math.log(2.0) * B_FULL * NSC)
    return np.float32(loss)


# revision 21
# speedup vs baseline: 1.2081x; 1.2081x over previous
"""Trainium2 Bass kernel for nn_DL_R_sum_MRC (MIMO MRC rate-sum loss).

Math (per batch b, RB i, subcarrier j, user k), derived from reference:
  V[c,t]   : unnormalized complex precoder (from y_pred), per (b, i)
  N2[c]    = sum_t |V[c,t]|^2           (normalization folded into the logs)
  hv[r,c]  = sum_t H_k[t,r] * V[c,t]    (complex, unnormalized)
  HF = hv[:,k], G = hv[:,1-k]
  q_u  = sum_r |HF_r|^2
  u_u  = sum_r conj(HF_r) * G_r
  DEN  = N2_k * (sigma * q_u * N2_kb + |u_u|^2 * P_kb)
  NUM  = DEN + q_u^2 * P_k * N2_kb
  rate = (ln NUM - ln DEN) / ln 2
  loss = -sum rate / (B * 52)

Sharding: pure data-parallel over batch, 8 NeuronCores x 512 batch.
Each core reduces its rates to a [128, NCHUNK] partial-sum tile; host sums.

Complex dot products via 3-mul Karatsuba (per H-element h=a+ib, V-elem
v=c+id): S1 = sum vre*(hre+him), S2 = sum hre*(vim-vre),
S3 = sum him*(vre+vim); hv_re = S1-S3, hv_im = S1+S2.

On-chip layouts (batch in partitions, 128 per chunk; offsets in elements):
  hraw (DMA):  (sc26, t32, r2, e2)   sc*128 + t*4 + r*2 + e      f32 half
  hk[k]:       (sc52, r2, m3, t32)   sc*192 + r*96 + m*32 + t    bf16
               m in {0: re, 1: im, 2: re+im}
  vk:          (s3, c2, i13, t32)    s*832 + c*416 + i*32 + t    bf16
               s in {0: vre, 1: vim-vre, 2: vre+vim}
  vsq:         (s'2, c, i, t)        mirror of vk s1..s2          f32 (x0.5)
  pr:          (q=(i,j,r,c), t32)    q*32 + t                     bf16
  pl1..pl4:    (q, t16/8/4/2)
  sk[k]:       (m3, q208)            m*208 + q                    bf16
  hvall:       (part2, k2, i, jr, c) part*416 + k*208 + i*16+j*4.. wait
               q = i*16 + j*4 + r*2 + c ; hv uses (i, jr=(j,r), c) = q   f32
"""

import math
import sys

import numpy as np

sys.path.insert(0, "/opt/trn_rl_repo")

B_FULL = 4096
N_CORES = 8
NB = B_FULL // N_CORES  # 512 batch per core
P = 128                 # partitions per chunk
NCHUNK = NB // P        # 4 chunks
SIGMA = 0.1
NRB = 13
NSC = 52

H_FREE = NSC * 32 * 2 * 2   # 6656
Y_FREE = 64 * NRB * 2       # 1664
P_FREE = NRB * 2            # 26: (i, c)

_TRACE = {"on": False, "result": None}

# engine assignment: "v" = DVE (vector), "g" = GpSimd (Pool)
MUL_ENG = {(0, 0): "v", (0, 1): "v", (1, 0): "v", (1, 1): "v",
           (2, 0): "v", (2, 1): "v"}
TREE_ENG = {(0, 0): "v", (0, 1): "v", (1, 0): "v", (1, 1): "v",
            (2, 0): "v", (2, 1): "v"}
TAIL_ENG = {(0, 0): "g", (0, 1): "v", (1, 0): "g", (1, 1): "v",
            (2, 0): "g", (2, 1): "v"}
L2_TAIL = {(0, 0), (1, 0), (2, 0)}
HSUM_ENG = "g"
VPREP_ENG = "g"
UMUL_ENG = "v"


def _ap(x, off, dims):
    """View of tile/dram AP `x` at element offset `off` with free dims [[step, count], ...]."""
    import concourse.bass as bass

    return bass.AP(tensor=x.tensor, offset=x.offset + off, ap=[list(x.ap[0])] + dims)


def _build(nc, repeat=1, parts="all"):
    from contextlib import ExitStack

    import concourse.tile as tile
    from concourse import mybir

    f32 = mybir.dt.float32
    bf16 = mybir.dt.bfloat16
    Alu = mybir.AluOpType
    Act = mybir.ActivationFunctionType
    Ax = mybir.AxisListType

    h1d = nc.dram_tensor("h1", [NB, H_FREE], f32, kind="ExternalInput").ap()
    h2d = nc.dram_tensor("h2", [NB, H_FREE], f32, kind="ExternalInput").ap()
    yd = nc.dram_tensor("yp", [NB, Y_FREE], f32, kind="ExternalInput").ap()
    pd = nc.dram_tensor("pm", [NB, P_FREE], f32, kind="ExternalInput").ap()
    outd = nc.dram_tensor("partial", [P, NCHUNK], f32, kind="ExternalOutput").ap()

    def eng(which):
        return nc.vector if which == "v" else nc.gpsimd

    with tile.TileContext(nc) as tc, ExitStack() as ctx:
        hpool = ctx.enter_context(tc.tile_pool(name="hpool", bufs=2))
        hkpool = ctx.enter_context(tc.tile_pool(name="hkpool", bufs=2))
        vpool = ctx.enter_context(tc.tile_pool(name="vpool", bufs=2))
        prpool = ctx.enter_context(tc.tile_pool(name="prpool", bufs=2))
        plpool = ctx.enter_context(tc.tile_pool(name="plpool", bufs=2))
        pltail = ctx.enter_context(tc.tile_pool(name="pltail", bufs=3))
        skpool = ctx.enter_context(tc.tile_pool(name="skpool", bufs=2))
        epool = ctx.enter_context(tc.tile_pool(name="epool", bufs=2))
        persist = ctx.enter_context(tc.tile_pool(name="persist", bufs=1))

        racc = persist.tile([P, NCHUNK], f32)

        def emit_epilogue(ch, hvall, n2, pt):
            # ---- epilogue, k-interleaved stages (runs during next chunk) ----
            rsum = epool.tile([P, 52], f32, tag="rsum")
            ijr = [[16, 13], [2, 8]]  # (i, jr) views into hvall
            rfold_a = [[2, 52]]       # (ij) at r=0, stride 2
            red_out = [[1, 52]]       # (ij)
            hfre, hfim, gre, gim = {}, {}, {}, {}
            t1, t2, qu, ure, uim, uu2, den, num = ({} for _ in range(8))
            for k in range(2):
                kb = 1 - k
                hfre[k] = _ap(hvall, k * 208 + k, ijr)
                hfim[k] = _ap(hvall, 416 + k * 208 + k, ijr)
                gre[k] = _ap(hvall, k * 208 + kb, ijr)
                gim[k] = _ap(hvall, 416 + k * 208 + kb, ijr)
                t1[k] = epool.tile([P, 104], f32, tag=f"t1{k}", name=f"t1{k}")
                t2[k] = epool.tile([P, 104], f32, tag=f"t2{k}", name=f"t2{k}")
                qu[k] = epool.tile([P, 52], f32, tag=f"qu{k}", name=f"qu{k}")
                ure[k] = epool.tile([P, 52], f32, tag=f"ure{k}", name=f"ure{k}")
                uim[k] = epool.tile([P, 52], f32, tag=f"uim{k}", name=f"uim{k}")
                uu2[k] = epool.tile([P, 52], f32, tag=f"uu2{k}", name=f"uu2{k}")
                den[k] = epool.tile([P, 52], f32, tag=f"den{k}", name=f"den{k}")
                num[k] = epool.tile([P, 52], f32, tag=f"num{k}", name=f"num{k}")

            # q_u = sum_r hfre^2 + hfim^2
            for k in range(2):
                nc.scalar.activation(t1[k], hfre[k], Act.Square)
                nc.scalar.activation(t2[k], hfim[k], Act.Square)
            for k in range(2):
                nc.vector.tensor_add(t1[k], t1[k], t2[k])
            for k in range(2):
                nc.vector.tensor_add(_ap(qu[k], 0, red_out),
                                     _ap(t1[k], 0, rfold_a),
                                     _ap(t1[k], 1, rfold_a))
            # u_re = sum_r hfre*gre + hfim*gim
            for k in range(2):
                eng(UMUL_ENG).tensor_mul(t1[k], hfre[k], gre[k])
                eng(UMUL_ENG).tensor_mul(t2[k], hfim[k], gim[k])
            for k in range(2):
                nc.vector.tensor_add(t1[k], t1[k], t2[k])
                nc.vector.tensor_add(_ap(ure[k], 0, red_out),
                                     _ap(t1[k], 0, rfold_a),
                                     _ap(t1[k], 1, rfold_a))
            # u_im = sum_r hfre*gim - hfim*gre
            for k in range(2):
                eng(UMUL_ENG).tensor_mul(t1[k], hfre[k], gim[k])
                eng(UMUL_ENG).tensor_mul(t2[k], hfim[k], gre[k])
            for k in range(2):
                nc.vector.tensor_sub(t1[k], t1[k], t2[k])
                nc.vector.tensor_add(_ap(uim[k], 0, red_out),
                                     _ap(t1[k], 0, rfold_a),
                                     _ap(t1[k], 1, rfold_a))
            # |u|^2
            for k in range(2):
                nc.scalar.activation(ure[k], ure[k], Act.Square)
                nc.scalar.activation(uim[k], uim[k], Act.Square)
            for k in range(2):
                nc.vector.tensor_add(uu2[k], ure[k], uim[k])

            for k in range(2):
                kb = 1 - k
                bkb = _ap(n2, 13 * kb, [[1, 13], [0, 4]])
                pkb = _ap(pt, kb, [[2, 13], [0, 4]])
                # den = bk * (sigma*qu*bkb + uu2*pkb)
                nc.vector.scalar_tensor_tensor(
                    out=den[k], in0=qu[k], scalar=SIGMA, in1=bkb,
                    op0=Alu.mult, op1=Alu.mult)
                nc.vector.tensor_mul(num[k], uu2[k], pkb)  # num as scratch
            for k in range(2):
                bk = _ap(n2, 13 * k, [[1, 13], [0, 4]])
                nc.vector.tensor_add(den[k], den[k], num[k])
                nc.vector.tensor_mul(den[k], den[k], bk)
            for k in range(2):
                kb = 1 - k
                bkb = _ap(n2, 13 * kb, [[1, 13], [0, 4]])
                pk = _ap(pt, k, [[2, 13], [0, 4]])
                # num = den + qu^2 * pk * bkb
                nc.vector.tensor_mul(num[k], qu[k], qu[k])
                nc.vector.tensor_mul(num[k], num[k], pk)
                nc.vector.tensor_mul(num[k], num[k], bkb)
                nc.vector.tensor_add(num[k], num[k], den[k])
            for k in range(2):
                nc.scalar.activation(den[k], den[k], Act.Ln)
                nc.scalar.activation(num[k], num[k], Act.Ln)
            nc.vector.tensor_sub(rsum, num[0], den[0])
            nc.vector.tensor_sub(num[1], num[1], den[1])
            nc.vector.tensor_add(rsum, rsum, num[1])

            nc.vector.tensor_reduce(
                out=_ap(racc, ch, [[1, 1]]),
                in_=rsum, axis=Ax.X, op=Alu.add)

        pending = None
        for ch in [c for _ in range(repeat) for c in range(NCHUNK)]:
            b0 = ch * P
            bsl = slice(b0, b0 + P)

            yt = vpool.tile([P, Y_FREE], f32, tag="yt")
            nc.sync.dma_start(out=yt, in_=yd[bsl, :])
            pt = vpool.tile([P, P_FREE], f32, tag="pt")
            nc.sync.dma_start(out=pt, in_=pd[bsl, :])

            # ---- V prep: vk (s, c, i, t) ----
            vk = vpool.tile([P, 3 * 832], bf16, tag="vk")
            vdim = [[416, 2], [32, NRB], [1, 32]]
            # s0: vre
            nc.scalar.copy(_ap(vk, 0, vdim),
                           _ap(yt, 0, [[832, 2], [2, NRB], [26, 32]]))
            yre = _ap(yt, 0, [[832, 2], [2, NRB], [26, 32]])
            yim = _ap(yt, 1, [[832, 2], [2, NRB], [26, 32]])
            # s1: vim - vre ; s2: vre + vim
            eng(VPREP_ENG).tensor_sub(_ap(vk, 832, vdim), yim, yre)
            eng(VPREP_ENG).tensor_add(_ap(vk, 1664, vdim), yre, yim)

            # ---- H relayout + cast: hraw (sc,t,r,e) -> hk (sc, r, m, t) ----
            hks = []
            for k, hd in ((0, h1d), (1, h2d)):
                hk = hkpool.tile([P, NSC * 192], bf16, tag=f"hk{k}")
                hks.append(hk)
                for half in range(2):
                    off = half * 26 * 128
                    hraw = hpool.tile([P, H_FREE // 2], f32, tag="hraw",
                                      name=f"hraw{k}{half}")
                    nc.sync.dma_start(out=hraw, in_=hd[bsl, off:off + 3328])
                    for e in range(2):
                        nc.scalar.copy(
                            _ap(hk, half * 26 * 192 + e * 32,
                                [[192, 26], [96, 2], [1, 32]]),
                            _ap(hraw, e, [[128, 26], [2, 2], [4, 32]]),
                        )
                # hsum slot m=2: re + im
                eng(HSUM_ENG).tensor_add(
                    _ap(hk, 64, [[192, NSC], [96, 2], [1, 32]]),
                    _ap(hk, 0, [[192, NSC], [96, 2], [1, 32]]),
                    _ap(hk, 32, [[192, NSC], [96, 2], [1, 32]]),
                )

            if parts == "dmaonly":
                nc.vector.tensor_copy(_ap(racc, ch, [[1, 1]]),
                                      _ap(hks[0], 0, [[1, 1]]))
                continue

            # previous chunk's epilogue: emitted here so it executes while
            # this chunk's streams flow, off the critical path
            if pending is not None:
                emit_epilogue(*pending)
                pending = None

            # ---- products + tree per (k, m-stream) ----
            # stream m: (h-slot, v-slot): m0: (hsum=2, vre=0) -> S1
            #           m1: (hre=0, vd1=1) -> S2 ; m2: (him=1, vd2=2) -> S3
            hvall = epool.tile([P, 832], bf16, tag="hvall")
            sks = []
            for k in range(2):
                hk = hks[k]
                sk = skpool.tile([P, 3 * 208], bf16, tag=f"sk{k}")
                sks.append(sk)
                for m, (hm, vs) in ((1, (0, 1)), (2, (1, 2)), (0, (2, 0))):
                    me = eng(MUL_ENG[(m, k)])
                    te = eng(TREE_ENG[(m, k)])
                    pr = prpool.tile([P, 6656], bf16, tag="pr")
                    for c in range(2):
                        # (j, r) merged: h stride 96 x8, pr stride 64 x8
                        me.tensor_mul(
                            _ap(pr, c * 32,
                                [[512, NRB], [64, 8], [1, 32]]),
                            _ap(hk, hm * 32,
                                [[768, NRB], [96, 8], [1, 32]]),
                            _ap(vk, vs * 832 + c * 416,
                                [[32, NRB], [0, 8], [1, 32]]),
                        )
                    pl1 = plpool.tile([P, 3328], bf16, tag="pl1")
                    te.tensor_add(_ap(pl1, 0, [[16, 208], [1, 16]]),
                                  _ap(pr, 0, [[32, 208], [1, 16]]),
                                  _ap(pr, 16, [[32, 208], [1, 16]]))
                    tl = eng(TAIL_ENG[(m, k)])
                    l2e = tl if (m, k) in L2_TAIL else te
                    pl2 = plpool.tile([P, 1664], bf16, tag="pl2")
                    l2e.tensor_add(_ap(pl2, 0, [[8, 208], [1, 8]]),
                                  _ap(pl1, 0, [[16, 208], [1, 8]]),
                                  _ap(pl1, 8, [[16, 208], [1, 8]]))
                    pl3 = pltail.tile([P, 832], bf16, tag="pl3")
                    tl.tensor_add(_ap(pl3, 0, [[4, 208], [1, 4]]),
                                  _ap(pl2, 0, [[8, 208], [1, 4]]),
                                  _ap(pl2, 4, [[8, 208], [1, 4]]))
                    pl4 = pltail.tile([P, 416], bf16, tag="pl4")
                    tl.tensor_add(_ap(pl4, 0, [[2, 208], [1, 2]]),
                                  _ap(pl3, 0, [[4, 208], [1, 2]]),
                                  _ap(pl3, 2, [[4, 208], [1, 2]]))
                    tl.tensor_add(_ap(sk, m * 208, [[1, 208]]),
                                  _ap(pl4, 0, [[2, 208]]),
                                  _ap(pl4, 1, [[2, 208]]))
            # combines after all streams so the DVE queue never stalls on a
            # cross-engine tree mid-chunk
            for k in range(2):
                sk = sks[k]
                # combine: hv_re = S1 - S3 -> part 0; hv_im = S1 + S2 -> part 1
                nc.vector.tensor_sub(_ap(hvall, k * 208, [[1, 208]]),
                                     _ap(sk, 0, [[1, 208]]),
                                     _ap(sk, 416, [[1, 208]]))
                nc.vector.tensor_add(_ap(hvall, 416 + k * 208, [[1, 208]]),
                                     _ap(sk, 0, [[1, 208]]),
                                     _ap(sk, 208, [[1, 208]]))

            if parts == "prodonly":
                nc.vector.tensor_copy(_ap(racc, ch, [[1, 1]]),
                                      _ap(hvall, 0, [[1, 1]]))
                continue

            # ---- N2[c,i] = sum_t vre^2+vim^2 = sum_{s',t} vk[s1..s2]^2 / 2
            vsq = persist.tile([P, 1664], bf16, tag="vsq")
            nc.scalar.activation(vsq, _ap(vk, 832, [[1, 1664]]), Act.Square,
                                 scale=float(1.0 / math.sqrt(2.0)))
            n2s = epool.tile([P, 52], f32, tag="n2s")  # (s', ci)
            nc.vector.tensor_reduce(
                out=_ap(n2s, 0, [[1, 52]]),
                in_=_ap(vsq, 0, [[832, 2], [32, 26], [1, 32]]),
                axis=Ax.X, op=Alu.add)
            n2 = epool.tile([P, 26], f32, tag="n2")  # (c, i): c*13+i
            nc.vector.tensor_add(n2, _ap(n2s, 0, [[1, 26]]),
                                 _ap(n2s, 26, [[1, 26]]))


            pending = (ch, hvall, n2, pt)

        if pending is not None:
            emit_epilogue(*pending)

        nc.sync.dma_start(out=outd, in_=racc)

    return nc


def _make_program(repeat=1):
    from concourse import bacc

    nc = bacc.Bacc("TRN2", target_bir_lowering=False, debug=False,
                   num_devices=N_CORES)
    _build(nc, repeat=repeat)
    nc.compile()
    return nc


def kernel(H_dl_RB_1, H_dl_RB_2, P_marix, y_pred):
    from concourse.bass_utils import run_bass_kernel_spmd

    h1 = np.ascontiguousarray(np.asarray(H_dl_RB_1, dtype=np.float32)).reshape(B_FULL, H_FREE)
    h2 = np.ascontiguousarray(np.asarray(H_dl_RB_2, dtype=np.float32)).reshape(B_FULL, H_FREE)
    yp = np.ascontiguousarray(np.asarray(y_pred, dtype=np.float32)).reshape(B_FULL, Y_FREE)
    pm = np.ascontiguousarray(np.asarray(P_marix, dtype=np.float32)).reshape(B_FULL, P_FREE)

    nc = _make_program()
    in_maps = []
    for c in range(N_CORES):
        s = slice(c * NB, (c + 1) * NB)
        in_maps.append({"h1": h1[s], "h2": h2[s], "yp": yp[s], "pm": pm[s]})

    res = run_bass_kernel_spmd(nc, in_maps, list(range(N_CORES)),
                               trace=_TRACE["on"])
    _TRACE["result"] = res
    total = np.float64(0.0)
    for r in res.results:
        total += np.float64(r["partial"].astype(np.float64).sum())
    loss = -total / (math.log(2.0) * B_FULL * NSC)
    return np.float32(loss)


# revision 22
# speedup vs baseline: 1.5325x; 1.2685x over previous
"""Trainium2 Bass kernel for nn_DL_R_sum_MRC (MIMO MRC rate-sum loss).

Math (per batch b, RB i, subcarrier j, user k), derived from reference:
  V[c,t]   : unnormalized complex precoder (from y_pred), per (b, i)
  N2[c]    = sum_t |V[c,t]|^2           (normalization folded into the logs)
  hv[r,c]  = sum_t H_k[t,r] * V[c,t]    (complex, unnormalized)
  HF = hv[:,k], G = hv[:,1-k]
  q_u  = sum_r |HF_r|^2
  u_u  = sum_r conj(HF_r) * G_r
  DEN  = N2_k * (sigma * q_u * N2_kb + |u_u|^2 * P_kb)
  NUM  = DEN + q_u^2 * P_k * N2_kb
  rate = (ln NUM - ln DEN) / ln 2
  loss = -sum rate / (B * 52)

Sharding: pure data-parallel over batch, 8 NeuronCores x 512 batch.
Each core reduces its rates to a [128, NCHUNK] partial-sum tile; host sums.

Complex dot products via 3-mul Karatsuba (per H-element h=a+ib, V-elem
v=c+id): S1 = sum vre*(hre+him), S2 = sum hre*(vim-vre),
S3 = sum him*(vre+vim); hv_re = S1-S3, hv_im = S1+S2.

On-chip layouts (batch in partitions, 128 per chunk; offsets in elements):
  hraw (DMA):  (sc26, t32, r2, e2)   sc*128 + t*4 + r*2 + e      f32 half
  hk[k]:       (sc52, r2, m3, t32)   sc*192 + r*96 + m*32 + t    bf16
               m in {0: re, 1: im, 2: re+im}
  vk:          (s3, c2, i13, t32)    s*832 + c*416 + i*32 + t    bf16
               s in {0: vre, 1: vim-vre, 2: vre+vim}
  vsq:         (s'2, c, i, t)        mirror of vk s1..s2          f32 (x0.5)
  pr:          (q=(i,j,r,c), t32)    q*32 + t                     bf16
  pl1..pl4:    (q, t16/8/4/2)
  sk[k]:       (m3, q208)            m*208 + q                    bf16
  hvall:       (part2, k2, i, jr, c) part*416 + k*208 + i*16+j*4.. wait
               q = i*16 + j*4 + r*2 + c ; hv uses (i, jr=(j,r), c) = q   f32
"""

import math
import sys

import numpy as np

sys.path.insert(0, "/opt/trn_rl_repo")

B_FULL = 4096
N_CORES = 8
NB = B_FULL // N_CORES  # 512 batch per core
P = 128                 # partitions per chunk
NCHUNK = NB // P        # 4 chunks
SIGMA = 0.1
NRB = 13
NSC = 52

H_FREE = NSC * 32 * 2 * 2   # 6656
Y_FREE = 64 * NRB * 2       # 1664
P_FREE = NRB * 2            # 26: (i, c)

_TRACE = {"on": False, "result": None}

# engine assignment: "v" = DVE (vector), "g" = GpSimd (Pool)
MUL_ENG = {(0, 0): "v", (0, 1): "v", (1, 0): "v", (1, 1): "v",
           (2, 0): "v", (2, 1): "v"}
TREE_ENG = {(0, 0): "v", (0, 1): "v", (1, 0): "v", (1, 1): "v",
            (2, 0): "v", (2, 1): "v"}
TAIL_ENG = {(0, 0): "v", (0, 1): "v", (1, 0): "v", (1, 1): "v",
            (2, 0): "v", (2, 1): "v"}
L2_TAIL = set()
HSUM_ENG = "v"
VPREP_ENG = "v"
UMUL_ENG = "v"


def _ap(x, off, dims):
    """View of tile/dram AP `x` at element offset `off` with free dims [[step, count], ...]."""
    import concourse.bass as bass

    return bass.AP(tensor=x.tensor, offset=x.offset + off, ap=[list(x.ap[0])] + dims)


def _build(nc, repeat=1, parts="all"):
    from contextlib import ExitStack

    import concourse.tile as tile
    from concourse import mybir

    f32 = mybir.dt.float32
    bf16 = mybir.dt.bfloat16
    Alu = mybir.AluOpType
    Act = mybir.ActivationFunctionType
    Ax = mybir.AxisListType

    h1d = nc.dram_tensor("h1", [NB, H_FREE], f32, kind="ExternalInput").ap()
    h2d = nc.dram_tensor("h2", [NB, H_FREE], f32, kind="ExternalInput").ap()
    yd = nc.dram_tensor("yp", [NB, Y_FREE], f32, kind="ExternalInput").ap()
    pd = nc.dram_tensor("pm", [NB, P_FREE], f32, kind="ExternalInput").ap()
    outd = nc.dram_tensor("partial", [P, NCHUNK], f32, kind="ExternalOutput").ap()

    def eng(which):
        return nc.vector if which == "v" else nc.gpsimd

    with tile.TileContext(nc) as tc, ExitStack() as ctx:
        hpool = ctx.enter_context(tc.tile_pool(name="hpool", bufs=2))
        hkpool = ctx.enter_context(tc.tile_pool(name="hkpool", bufs=2))
        vpool = ctx.enter_context(tc.tile_pool(name="vpool", bufs=2))
        prpool = ctx.enter_context(tc.tile_pool(name="prpool", bufs=2))
        plpool = ctx.enter_context(tc.tile_pool(name="plpool", bufs=2))
        pltail = ctx.enter_context(tc.tile_pool(name="pltail", bufs=3))
        skpool = ctx.enter_context(tc.tile_pool(name="skpool", bufs=2))
        epool = ctx.enter_context(tc.tile_pool(name="epool", bufs=2))
        persist = ctx.enter_context(tc.tile_pool(name="persist", bufs=1))

        racc = persist.tile([P, NCHUNK], f32)

        def emit_epilogue(ch, hvall, n2, pt):
            # ---- epilogue, k-interleaved stages (runs during next chunk) ----
            rsum = epool.tile([P, 52], f32, tag="rsum")
            ijr = [[16, 13], [2, 8]]  # (i, jr) views into hvall
            rfold_a = [[2, 52]]       # (ij) at r=0, stride 2
            red_out = [[1, 52]]       # (ij)
            hfre, hfim, gre, gim = {}, {}, {}, {}
            t1, t2, qu, ure, uim, uu2, den, num = ({} for _ in range(8))
            for k in range(2):
                kb = 1 - k
                hfre[k] = _ap(hvall, k * 208 + k, ijr)
                hfim[k] = _ap(hvall, 416 + k * 208 + k, ijr)
                gre[k] = _ap(hvall, k * 208 + kb, ijr)
                gim[k] = _ap(hvall, 416 + k * 208 + kb, ijr)
                t1[k] = epool.tile([P, 104], f32, tag=f"t1{k}", name=f"t1{k}")
                t2[k] = epool.tile([P, 104], f32, tag=f"t2{k}", name=f"t2{k}")
                qu[k] = epool.tile([P, 52], f32, tag=f"qu{k}", name=f"qu{k}")
                ure[k] = epool.tile([P, 52], f32, tag=f"ure{k}", name=f"ure{k}")
                uim[k] = epool.tile([P, 52], f32, tag=f"uim{k}", name=f"uim{k}")
                uu2[k] = epool.tile([P, 52], f32, tag=f"uu2{k}", name=f"uu2{k}")
                den[k] = epool.tile([P, 52], f32, tag=f"den{k}", name=f"den{k}")
                num[k] = epool.tile([P, 52], f32, tag=f"num{k}", name=f"num{k}")

            # q_u = sum_r hfre^2 + hfim^2
            for k in range(2):
                nc.scalar.activation(t1[k], hfre[k], Act.Square)
                nc.scalar.activation(t2[k], hfim[k], Act.Square)
            for k in range(2):
                nc.vector.tensor_add(t1[k], t1[k], t2[k])
            for k in range(2):
                nc.vector.tensor_add(_ap(qu[k], 0, red_out),
                                     _ap(t1[k], 0, rfold_a),
                                     _ap(t1[k], 1, rfold_a))
            # u_re = sum_r hfre*gre + hfim*gim
            for k in range(2):
                eng(UMUL_ENG).tensor_mul(t1[k], hfre[k], gre[k])
                eng(UMUL_ENG).tensor_mul(t2[k], hfim[k], gim[k])
            for k in range(2):
                nc.vector.tensor_add(t1[k], t1[k], t2[k])
                nc.vector.tensor_add(_ap(ure[k], 0, red_out),
                                     _ap(t1[k], 0, rfold_a),
                                     _ap(t1[k], 1, rfold_a))
            # u_im = sum_r hfre*gim - hfim*gre
            for k in range(2):
                eng(UMUL_ENG).tensor_mul(t1[k], hfre[k], gim[k])
                eng(UMUL_ENG).tensor_mul(t2[k], hfim[k], gre[k])
            for k in range(2):
                nc.vector.tensor_sub(t1[k], t1[k], t2[k])
                nc.vector.tensor_add(_ap(uim[k], 0, red_out),
                                     _ap(t1[k], 0, rfold_a),
                                     _ap(t1[k], 1, rfold_a))
            # |u|^2
            for k in range(2):
                nc.scalar.activation(ure[k], ure[k], Act.Square)
                nc.scalar.activation(uim[k], uim[k], Act.Square)
            for k in range(2):
                nc.vector.tensor_add(uu2[k], ure[k], uim[k])

            for k in range(2):
                kb = 1 - k
                bkb = _ap(n2, 13 * kb, [[1, 13], [0, 4]])
                pkb = _ap(pt, kb, [[2, 13], [0, 4]])
                # den = bk * (sigma*qu*bkb + uu2*pkb)
                nc.vector.scalar_tensor_tensor(
                    out=den[k], in0=qu[k], scalar=SIGMA, in1=bkb,
                    op0=Alu.mult, op1=Alu.mult)
                nc.vector.tensor_mul(num[k], uu2[k], pkb)  # num as scratch
            for k in range(2):
                bk = _ap(n2, 13 * k, [[1, 13], [0, 4]])
                nc.vector.tensor_add(den[k], den[k], num[k])
                nc.vector.tensor_mul(den[k], den[k], bk)
            for k in range(2):
                kb = 1 - k
                bkb = _ap(n2, 13 * kb, [[1, 13], [0, 4]])
                pk = _ap(pt, k, [[2, 13], [0, 4]])
                # num = den + qu^2 * pk * bkb
                nc.vector.tensor_mul(num[k], qu[k], qu[k])
                nc.vector.tensor_mul(num[k], num[k], pk)
                nc.vector.tensor_mul(num[k], num[k], bkb)
                nc.vector.tensor_add(num[k], num[k], den[k])
            for k in range(2):
                nc.scalar.activation(den[k], den[k], Act.Ln)
                nc.scalar.activation(num[k], num[k], Act.Ln)
            nc.vector.tensor_sub(rsum, num[0], den[0])
            nc.vector.tensor_sub(num[1], num[1], den[1])
            nc.vector.tensor_add(rsum, rsum, num[1])

            nc.vector.tensor_reduce(
                out=_ap(racc, ch, [[1, 1]]),
                in_=rsum, axis=Ax.X, op=Alu.add)

        pending = None
        for ch in [c for _ in range(repeat) for c in range(NCHUNK)]:
            b0 = ch * P
            bsl = slice(b0, b0 + P)

            yt = vpool.tile([P, Y_FREE], f32, tag="yt")
            nc.sync.dma_start(out=yt, in_=yd[bsl, :])
            pt = vpool.tile([P, P_FREE], f32, tag="pt")
            nc.sync.dma_start(out=pt, in_=pd[bsl, :])

            # ---- V prep: vk (s, c, i, t) ----
            vk = vpool.tile([P, 3 * 832], bf16, tag="vk")
            vdim = [[416, 2], [32, NRB], [1, 32]]
            # s0: vre
            nc.scalar.copy(_ap(vk, 0, vdim),
                           _ap(yt, 0, [[832, 2], [2, NRB], [26, 32]]))
            yre = _ap(yt, 0, [[832, 2], [2, NRB], [26, 32]])
            yim = _ap(yt, 1, [[832, 2], [2, NRB], [26, 32]])
            # s1: vim - vre ; s2: vre + vim
            eng(VPREP_ENG).tensor_sub(_ap(vk, 832, vdim), yim, yre)
            eng(VPREP_ENG).tensor_add(_ap(vk, 1664, vdim), yre, yim)

            # ---- H relayout + cast: hraw (sc,t,r,e) -> hk (sc, r, m, t) ----
            hks = []
            for k, hd in ((0, h1d), (1, h2d)):
                hk = hkpool.tile([P, NSC * 192], bf16, tag=f"hk{k}")
                hks.append(hk)
                for half in range(2):
                    off = half * 26 * 128
                    hraw = hpool.tile([P, H_FREE // 2], f32, tag="hraw",
                                      name=f"hraw{k}{half}")
                    nc.sync.dma_start(out=hraw, in_=hd[bsl, off:off + 3328])
                    for e in range(2):
                        nc.scalar.copy(
                            _ap(hk, half * 26 * 192 + e * 32,
                                [[192, 26], [96, 2], [1, 32]]),
                            _ap(hraw, e, [[128, 26], [2, 2], [4, 32]]),
                        )
                # hsum slot m=2: re + im
                eng(HSUM_ENG).tensor_add(
                    _ap(hk, 64, [[192, NSC], [96, 2], [1, 32]]),
                    _ap(hk, 0, [[192, NSC], [96, 2], [1, 32]]),
                    _ap(hk, 32, [[192, NSC], [96, 2], [1, 32]]),
                )

            if parts == "dmaonly":
                nc.vector.tensor_copy(_ap(racc, ch, [[1, 1]]),
                                      _ap(hks[0], 0, [[1, 1]]))
                continue

            # previous chunk's epilogue: emitted here so it executes while
            # this chunk's streams flow, off the critical path
            if pending is not None:
                emit_epilogue(*pending)
                pending = None

            # ---- products + tree per (k, m-stream) ----
            # stream m: (h-slot, v-slot): m0: (hsum=2, vre=0) -> S1
            #           m1: (hre=0, vd1=1) -> S2 ; m2: (him=1, vd2=2) -> S3
            hvall = epool.tile([P, 832], bf16, tag="hvall")
            sks = []
            for k in range(2):
                hk = hks[k]
                sk = skpool.tile([P, 3 * 208], bf16, tag=f"sk{k}")
                sks.append(sk)
                for m, (hm, vs) in ((1, (0, 1)), (2, (1, 2)), (0, (2, 0))):
                    me = eng(MUL_ENG[(m, k)])
                    te = eng(TREE_ENG[(m, k)])
                    pr = prpool.tile([P, 6656], bf16, tag="pr")
                    for c in range(2):
                        # (j, r) merged: h stride 96 x8, pr stride 64 x8
                        me.tensor_mul(
                            _ap(pr, c * 32,
                                [[512, NRB], [64, 8], [1, 32]]),
                            _ap(hk, hm * 32,
                                [[768, NRB], [96, 8], [1, 32]]),
                            _ap(vk, vs * 832 + c * 416,
                                [[32, NRB], [0, 8], [1, 32]]),
                        )
                    pl1 = plpool.tile([P, 3328], bf16, tag="pl1")
                    te.tensor_add(_ap(pl1, 0, [[16, 208], [1, 16]]),
                                  _ap(pr, 0, [[32, 208], [1, 16]]),
                                  _ap(pr, 16, [[32, 208], [1, 16]]))
                    tl = eng(TAIL_ENG[(m, k)])
                    l2e = tl if (m, k) in L2_TAIL else te
                    pl2 = plpool.tile([P, 1664], bf16, tag="pl2")
                    l2e.tensor_add(_ap(pl2, 0, [[8, 208], [1, 8]]),
                                  _ap(pl1, 0, [[16, 208], [1, 8]]),
                                  _ap(pl1, 8, [[16, 208], [1, 8]]))
                    pl3 = pltail.tile([P, 832], bf16, tag="pl3")
                    tl.tensor_add(_ap(pl3, 0, [[4, 208], [1, 4]]),
                                  _ap(pl2, 0, [[8, 208], [1, 4]]),
                                  _ap(pl2, 4, [[8, 208], [1, 4]]))
                    pl4 = pltail.tile([P, 416], bf16, tag="pl4")
                    tl.tensor_add(_ap(pl4, 0, [[2, 208], [1, 2]]),
                                  _ap(pl3, 0, [[4, 208], [1, 2]]),
                                  _ap(pl3, 2, [[4, 208], [1, 2]]))
                    tl.tensor_add(_ap(sk, m * 208, [[1, 208]]),
                                  _ap(pl4, 0, [[2, 208]]),
                                  _ap(pl4, 1, [[2, 208]]))
            # combines after all streams so the DVE queue never stalls on a
            # cross-engine tree mid-chunk
            for k in range(2):
                sk = sks[k]
                # combine: hv_re = S1 - S3 -> part 0; hv_im = S1 + S2 -> part 1
                nc.vector.tensor_sub(_ap(hvall, k * 208, [[1, 208]]),
                                     _ap(sk, 0, [[1, 208]]),
                                     _ap(sk, 416, [[1, 208]]))
                nc.vector.tensor_add(_ap(hvall, 416 + k * 208, [[1, 208]]),
                                     _ap(sk, 0, [[1, 208]]),
                                     _ap(sk, 208, [[1, 208]]))

            if parts == "prodonly":
                nc.vector.tensor_copy(_ap(racc, ch, [[1, 1]]),
                                      _ap(hvall, 0, [[1, 1]]))
                continue

            # ---- N2[c,i] = sum_t vre^2+vim^2 = sum_{s',t} vk[s1..s2]^2 / 2
            vsq = persist.tile([P, 1664], bf16, tag="vsq")
            nc.scalar.activation(vsq, _ap(vk, 832, [[1, 1664]]), Act.Square,
                                 scale=float(1.0 / math.sqrt(2.0)))
            n2s = epool.tile([P, 52], f32, tag="n2s")  # (s', ci)
            nc.vector.tensor_reduce(
                out=_ap(n2s, 0, [[1, 52]]),
                in_=_ap(vsq, 0, [[832, 2], [32, 26], [1, 32]]),
                axis=Ax.X, op=Alu.add)
            n2 = epool.tile([P, 26], f32, tag="n2")  # (c, i): c*13+i
            nc.vector.tensor_add(n2, _ap(n2s, 0, [[1, 26]]),
                                 _ap(n2s, 26, [[1, 26]]))


            pending = (ch, hvall, n2, pt)

        if pending is not None:
            emit_epilogue(*pending)

        nc.sync.dma_start(out=outd, in_=racc)

    return nc


def _make_program(repeat=1):
    from concourse import bacc

    nc = bacc.Bacc("TRN2", target_bir_lowering=False, debug=False,
                   num_devices=N_CORES)
    _build(nc, repeat=repeat)
    nc.compile()
    return nc


def kernel(H_dl_RB_1, H_dl_RB_2, P_marix, y_pred):
    from concourse.bass_utils import run_bass_kernel_spmd

    h1 = np.ascontiguousarray(np.asarray(H_dl_RB_1, dtype=np.float32)).reshape(B_FULL, H_FREE)
    h2 = np.ascontiguousarray(np.asarray(H_dl_RB_2, dtype=np.float32)).reshape(B_FULL, H_FREE)
    yp = np.ascontiguousarray(np.asarray(y_pred, dtype=np.float32)).reshape(B_FULL, Y_FREE)
    pm = np.ascontiguousarray(np.asarray(P_marix, dtype=np.float32)).reshape(B_FULL, P_FREE)

    nc = _make_program()
    in_maps = []
    for c in range(N_CORES):
        s = slice(c * NB, (c + 1) * NB)
        in_maps.append({"h1": h1[s], "h2": h2[s], "yp": yp[s], "pm": pm[s]})

    res = run_bass_kernel_spmd(nc, in_maps, list(range(N_CORES)),
                               trace=_TRACE["on"])
    _TRACE["result"] = res
    total = np.float64(0.0)
    for r in res.results:
        total += np.float64(r["partial"].astype(np.float64).sum())
    loss = -total / (math.log(2.0) * B_FULL * NSC)
    return np.float32(loss)
